# revision 4
# baseline (speedup 1.0000x reference)
"""HGCN layer kernel for Trainium2, 8 NeuronCores, row-sharded SPMD.

Reference computation (N=6144, D=512):
    type_sum_a = adj_a @ x ; type_sum_b = adj_b @ x
    attn_a = sigmoid(cat[ts_a, x] @ Wa.T + ba) ; attn_b likewise
    h = x @ W_sa ; s_l = h @ a_sa[:512] ; s_r = h @ a_sa[512:]
    scores[i,j] = s_l[i] + s_r[j]
    e = adj_a * exp(-leaky_relu(scores, 0.01)) ; attn = e / (rowsum(e)+1e-5)
    x_a = attn @ h ; x_b = adj_b @ (x @ W_gcnb) + b_gcnb
    out = sigmoid(attn_a * x_a + attn_b * x_b)

Kernel strategy (per core, NL=768 local rows, global j order everywhere):
  - R = [W_sa | W_gcnb | W_sa@a_l | W_sa@a_r | Wa1.T | Wb1.T | Wa2.T | Wb2.T]
    Phase A computes HX = x_local @ R for the LOCAL rows only, then an
    AllGather (h|xw bf16, stats f32) replicates all rows' h/xW/stats to
    every core. Gates reassociate (adj@x)@W1.T -> adj@(x@W1.T) so the NxN
    gate matmuls shrink to N-vector contractions.
  - e computed in transposed layout [j(part), i(free)] so it is directly
    the lhsT of the attention matmul; adjacency is passed pre-transposed,
    sliced to local output rows, global j order.
  - rowsum(e) via per-i N=1 matmul (ones rhs) against the already-loaded
    e weights; ga/gb gate contractions accumulate on the Vector engine
    (acc += adj_tile * v[j]) and partition-reduce via N=1 matmuls.
  - float32r Phase A matmuls (full fp32 rate at N>=256); bf16 elsewhere.
"""

import numpy as np
from contextlib import ExitStack

import concourse.bass as bass
import concourse.bacc as bacc
import concourse.mybir as mybir
import concourse.tile as tile

F32 = mybir.dt.float32
F32R = mybir.dt.float32r
BF16 = mybir.dt.bfloat16
AF = mybir.ActivationFunctionType
ALU = mybir.AluOpType

N_CORES = 8


def build_program(n, d, nl, ba, bb, dt_a=F32R, dt_bc=BF16):
    """Build the SPMD Bass program. Returns nc.

    n: total nodes, d: feature dim, nl: local rows per core.
    ba/bb: python-float gate biases (baked in).
    """
    JT = n // 128   # j tiles (contraction/node axis), global order
    LT = nl // 128  # local row tiles
    KT = d // 128   # feature k tiles
    NR = 2 * d + 8  # columns of R
    # stats cols: 0=s_l 1=s_r 2=zero 3=va 4=vb 5=wa2x 6=wb2x 7=pad

    nc = bacc.Bacc("TRN2", target_bir_lowering=False, debug=False,
                   num_devices=N_CORES)

    xt_dram = nc.dram_tensor("xt", [LT, KT, 128, 128], dt_a, kind="ExternalInput")
    r_dram = nc.dram_tensor("rmat", [KT, 128, NR], dt_a, kind="ExternalInput")
    adjat_dram = nc.dram_tensor("adjat", [JT, 128, nl], dt_bc, kind="ExternalInput")
    adjbt_dram = nc.dram_tensor("adjbt", [JT, 128, nl], dt_bc, kind="ExternalInput")
    bbias_dram = nc.dram_tensor("bbias", [128, d], F32, kind="ExternalInput")
    ident_dram = nc.dram_tensor("ident", [128, 128], F32, kind="ExternalInput")
    out_dram = nc.dram_tensor("out", [nl, d], F32, kind="ExternalOutput")

    ag_hx_in = nc.dram_tensor("ag_hx_in", [LT, 128, 2 * d], dt_bc)
    ag_hx_out = nc.dram_tensor("ag_hx_out", [JT, 128, 2 * d], dt_bc,
                               addr_space="Shared")
    ag_st_in = nc.dram_tensor("ag_st_in", [LT, 128, 8], F32)
    ag_st_out = nc.dram_tensor("ag_st_out", [JT, 128, 8], F32,
                               addr_space="Shared")

    def mm(out, lhsT, rhs, start, stop, skip_group_check=False):
        nc.tensor.matmul(out, lhsT, rhs, start=start, stop=stop,
                         skip_group_check=skip_group_check)

    with tile.TileContext(nc) as tc, ExitStack() as ctx:
        const = ctx.enter_context(tc.tile_pool(name="const", bufs=1))

        r_sb = const.tile([128, KT, NR], dt_a, tag="r")
        h_sb = const.tile([128, JT * d], dt_bc, tag="h")
        xw_sb = const.tile([128, JT * d], dt_bc, tag="xw")
        stats_sb = const.tile([128, JT * 8], F32, tag="stats")
        stats_loc = const.tile([128, LT * 8], F32, tag="statsloc")
        slb_sb = const.tile([128, nl], F32, tag="slb")
        xb_sb = const.tile([128, LT * d], F32, tag="xb")
        xa_sb = const.tile([128, LT * d], F32, tag="xa")
        bbias_sb = const.tile([128, d], F32, tag="bbias")
        ident_sb = const.tile([128, 128], F32, tag="ident")
        ones_bf = const.tile([128, 1], dt_bc, tag="ones_bf")
        ones_f32 = const.tile([128, 1], F32, tag="ones_f32")
        ones_row = const.tile([1, 128], F32, tag="ones_r")
        neg1 = const.tile([128, 1], F32, tag="neg1")
        ba_sb = const.tile([128, 1], F32, tag="ba")
        bb_sb = const.tile([128, 1], F32, tag="bb")
        sl_row = const.tile([1, nl], F32, tag="sl_row")
        ga_acc = const.tile([128, nl], F32, tag="ga_acc")
        gb_acc = const.tile([128, nl], F32, tag="gb_acc")
        gate_sb = const.tile([128, 4 * LT], F32, tag="gate")
        # gate_sb cols: [0:LT]=recip(rowsum), [LT:2LT]=sig_a, [2LT:3LT]=sig_b,
        # [3LT:4LT]=scratch

        for k in range(KT):
            nc.sync.dma_start(out=r_sb[:, k, :], in_=r_dram[k])
        nc.sync.dma_start(out=bbias_sb[:], in_=bbias_dram[:])
        nc.sync.dma_start(out=ident_sb[:], in_=ident_dram[:])
        nc.vector.memset(ones_f32[:], 1.0)
        nc.vector.tensor_copy(ones_bf[:], ones_f32[:])
        nc.vector.memset(ones_row[:], 1.0)
        nc.vector.memset(neg1[:], -1.0)
        nc.vector.memset(ba_sb[:], float(ba))
        nc.vector.memset(bb_sb[:], float(bb))
        nc.vector.memset(ga_acc[:], 0.0)
        nc.vector.memset(gb_acc[:], 0.0)

        # ---- Phase A: HX = x_local @ R (local rows only) ----
        with tc.tile_pool(name="xt_pool", bufs=3) as xtp, \
             tc.tile_pool(name="hx_out", bufs=3) as hxop, \
             tc.tile_pool(name="psA", bufs=2, space="PSUM") as psA:
            for m in range(LT):
                xt_t = xtp.tile([128, KT * 128], dt_a, tag="xt")
                for k in range(KT):
                    nc.sync.dma_start(out=xt_t[:, k * 128:(k + 1) * 128],
                                      in_=xt_dram[m, k])
                ph = psA.tile([128, d], F32, tag="ph")
                pw = psA.tile([128, d], F32, tag="pw")
                ps = psA.tile([128, 8], F32, tag="ps")
                for k in range(KT):
                    lhsT = xt_t[:, k * 128:(k + 1) * 128]
                    st, sp = (k == 0), (k == KT - 1)
                    mm(ph[:], lhsT, r_sb[:, k, 0:d], st, sp)
                    mm(pw[:], lhsT, r_sb[:, k, d:2 * d], st, sp)
                    mm(ps[:], lhsT, r_sb[:, k, 2 * d:NR], st, sp)
                hx_t = hxop.tile([128, 2 * d], dt_bc, tag="hx")
                nc.scalar.copy(hx_t[:, 0:d], ph[:])
                nc.scalar.copy(hx_t[:, d:2 * d], pw[:])
                nc.sync.dma_start(out=ag_hx_in[m], in_=hx_t[:])
                nc.vector.tensor_copy(stats_loc[:, m * 8:(m + 1) * 8], ps[:])
                nc.sync.dma_start(out=ag_st_in[m],
                                  in_=stats_loc[:, m * 8:(m + 1) * 8])

        # ---- AllGather h|xw (bf16) and stats (f32) across the 8 cores ----
        nc.gpsimd.collective_compute(
            "AllGather", mybir.AluOpType.bypass,
            replica_groups=[list(range(N_CORES))],
            ins=[ag_hx_in[:].opt()], outs=[ag_hx_out[:].opt()])
        nc.gpsimd.collective_compute(
            "AllGather", mybir.AluOpType.bypass,
            replica_groups=[list(range(N_CORES))],
            ins=[ag_st_in[:].opt()], outs=[ag_st_out[:].opt()])

        # Readback: full h/xw/stats into SBUF (pipelined with Phase B/C use)
        for j in range(JT):
            nc.sync.dma_start(out=h_sb[:, j * d:(j + 1) * d],
                              in_=ag_hx_out[j, :, 0:d])
            nc.sync.dma_start(out=xw_sb[:, j * d:(j + 1) * d],
                              in_=ag_hx_out[j, :, d:2 * d])
            nc.sync.dma_start(out=stats_sb[:, j * 8:(j + 1) * 8],
                              in_=ag_st_out[j])

        # ---- Phase A2: build SL broadcast [128, nl] from local s_l ----
        # (local stats only; runs during the AllGather)
        with tc.tile_pool(name="psA2", bufs=1, space="PSUM") as psA2:
            chunks = [(o, min(512, nl - o)) for o in range(0, nl, 512)]
            ptrs = [psA2.tile([1, w], F32, tag=f"psl{ci}", name=f"psl{ci}")
                    for ci, (o, w) in enumerate(chunks)]
            for t in range(LT):
                ci, off = divmod(t * 128, 512)
                mm(ptrs[ci][0:1, off:off + 128],
                   stats_loc[:, t * 8:t * 8 + 1], ident_sb[:], True, True)
            for ci, (o, w) in enumerate(chunks):
                nc.vector.tensor_copy(sl_row[0:1, o:o + w], ptrs[ci][0:1, :])
            for ci, (o, w) in enumerate(chunks):
                pb = psA2.tile([128, w], F32, tag="pslb")
                mm(pb[:], ones_row[:], sl_row[0:1, o:o + w], True, True)
                nc.vector.tensor_copy(slb_sb[:, o:o + w], pb[:])

        # ---- Phase B: x_b = adj_b @ xW ; gb via vector accumulate ----
        with tc.tile_pool(name="adjB", bufs=5) as adjp, \
             tc.tile_pool(name="psB", bufs=1, space="PSUM") as psB:
            pb_acc = [psB.tile([128, d], F32, tag=f"pb{i}", name=f"pb{i}")
                      for i in range(LT)]
            pgbT = psB.tile([128, 8], F32, tag="pgbT")
            for j in range(JT):
                at = adjp.tile([128, nl], dt_bc, tag="adj")
                nc.sync.dma_start(out=at[:], in_=adjbt_dram[j])
                xw_t = xw_sb[:, j * d:(j + 1) * d]
                st, sp = (j == 0), (j == JT - 1)
                for i in range(LT):
                    mm(pb_acc[i][:], at[:, i * 128:(i + 1) * 128], xw_t, st, sp)
                vb = stats_sb[:, j * 8 + 4:j * 8 + 5]
                nc.vector.scalar_tensor_tensor(gb_acc[:], at[:], vb, gb_acc[:],
                                               op0=ALU.mult, op1=ALU.add)
            for i in range(LT):
                nc.scalar.copy(xb_sb[:, i * d:(i + 1) * d], pb_acc[i][:])
            for i in range(LT):
                mm(pgbT[:, i:i + 1], gb_acc[:, i * 128:(i + 1) * 128],
                   ones_f32[:], i == 0, i == LT - 1, skip_group_check=True)
            nc.vector.tensor_copy(gate_sb[:, 2 * LT:3 * LT], pgbT[:, 0:LT])

        # ---- Phase C: e = adj_a * exp(-lrelu(s)); x_a = e^T.T @ h ----
        # rowsum via N=1 ones-matmul per i (weights already loaded);
        # ga via vector accumulate like gb.
        with tc.tile_pool(name="adjC", bufs=5) as adjp, \
             tc.tile_pool(name="ewC", bufs=4) as ewp, \
             tc.tile_pool(name="psC", bufs=1, space="PSUM") as psC:
            pc_acc = [psC.tile([128, d], F32, tag=f"pc{i}", name=f"pc{i}")
                      for i in range(LT)]
            prsT = psC.tile([128, 8], F32, tag="prsT")
            pgaT = psC.tile([128, 8], F32, tag="pgaT")
            for j in range(JT):
                at = adjp.tile([128, nl], dt_bc, tag="adj")
                nc.sync.dma_start(out=at[:], in_=adjat_dram[j])
                s_r = stats_sb[:, j * 8 + 1:j * 8 + 2]
                m_t = ewp.tile([128, nl], F32, tag="m")
                nc.vector.tensor_scalar_add(m_t[:], slb_sb[:], s_r)
                nc.vector.scalar_tensor_tensor(m_t[:], m_t[:], 0.01, m_t[:],
                                               op0=ALU.mult, op1=ALU.max)
                # w = exp(-m), in place
                nc.scalar.activation(m_t[:], m_t[:], AF.Exp, scale=neg1[:])
                e_t = ewp.tile([128, nl], dt_bc, tag="e")
                nc.gpsimd.tensor_tensor(e_t[:], m_t[:], at[:], op=ALU.mult)
                va = stats_sb[:, j * 8 + 3:j * 8 + 4]
                nc.vector.scalar_tensor_tensor(ga_acc[:], at[:], va, ga_acc[:],
                                               op0=ALU.mult, op1=ALU.add)
                st, sp = (j == 0), (j == JT - 1)
                for i in range(LT):
                    mm(pc_acc[i][:], e_t[:, i * 128:(i + 1) * 128],
                       h_sb[:, j * d:(j + 1) * d], st, sp)
                    mm(prsT[:, i:i + 1], e_t[:, i * 128:(i + 1) * 128],
                       ones_bf[:], st and i == 0, sp and i == LT - 1,
                       skip_group_check=True)
            for i in range(LT):
                nc.scalar.copy(xa_sb[:, i * d:(i + 1) * d], pc_acc[i][:])
            for i in range(LT):
                mm(pgaT[:, i:i + 1], ga_acc[:, i * 128:(i + 1) * 128],
                   ones_f32[:], i == 0, i == LT - 1, skip_group_check=True)
            nc.vector.tensor_copy(gate_sb[:, 3 * LT:4 * LT], prsT[:, 0:LT])
            nc.vector.tensor_copy(gate_sb[:, LT:2 * LT], pgaT[:, 0:LT])

        # ---- Phase D: gates + combine ----
        with tc.tile_pool(name="outD", bufs=2) as outp:
            for i in range(LT):
                # recip(rowsum + 1e-5)
                nc.vector.tensor_scalar_add(gate_sb[:, 3 * LT + i:3 * LT + i + 1],
                                            gate_sb[:, 3 * LT + i:3 * LT + i + 1],
                                            1e-5)
                nc.vector.reciprocal(gate_sb[:, i:i + 1],
                                     gate_sb[:, 3 * LT + i:3 * LT + i + 1])
                # sig_a = sigmoid(ga + wa2x + ba)
                nc.vector.tensor_tensor(gate_sb[:, LT + i:LT + i + 1],
                                        gate_sb[:, LT + i:LT + i + 1],
                                        stats_loc[:, i * 8 + 5:i * 8 + 6],
                                        op=ALU.add)
                nc.scalar.activation(gate_sb[:, LT + i:LT + i + 1],
                                     gate_sb[:, LT + i:LT + i + 1],
                                     AF.Sigmoid, bias=ba_sb[:])
                # sig_b = sigmoid(gb + wb2x + bb)
                nc.vector.tensor_tensor(gate_sb[:, 2 * LT + i:2 * LT + i + 1],
                                        gate_sb[:, 2 * LT + i:2 * LT + i + 1],
                                        stats_loc[:, i * 8 + 6:i * 8 + 7],
                                        op=ALU.add)
                nc.scalar.activation(gate_sb[:, 2 * LT + i:2 * LT + i + 1],
                                     gate_sb[:, 2 * LT + i:2 * LT + i + 1],
                                     AF.Sigmoid, bias=bb_sb[:])
            for i in range(LT):
                u_t = outp.tile([128, d], F32, tag="u")
                # u = sig_a * (x_a_raw * recip)
                nc.vector.tensor_scalar(u_t[:], xa_sb[:, i * d:(i + 1) * d],
                                        gate_sb[:, i:i + 1],
                                        gate_sb[:, LT + i:LT + i + 1],
                                        op0=ALU.mult, op1=ALU.mult)
                t_t = outp.tile([128, d], F32, tag="t")
                # t = x_b_raw + b_gcnb
                nc.vector.tensor_tensor(t_t[:], xb_sb[:, i * d:(i + 1) * d],
                                        bbias_sb[:], op=ALU.add)
                # y = sigmoid(t * sig_b + u)
                nc.vector.scalar_tensor_tensor(t_t[:], t_t[:],
                                               gate_sb[:, 2 * LT + i:2 * LT + i + 1],
                                               u_t[:], op0=ALU.mult, op1=ALU.add)
                y_t = outp.tile([128, d], F32, tag="y")
                nc.scalar.activation(y_t[:], t_t[:], AF.Sigmoid)
                nc.sync.dma_start(out=out_dram[i * 128:(i + 1) * 128, :],
                                  in_=y_t[:])

    nc.compile()
    return nc


def make_r_matrix(W_sa, a_sa, W_gcnb, Wa, Wb, d):
    cols = np.zeros((d, 8), dtype=np.float32)
    cols[:, 0] = W_sa @ a_sa[0, :d]
    cols[:, 1] = W_sa @ a_sa[0, d:]
    # col 2 stays zero
    cols[:, 3] = Wa[0, :d]
    cols[:, 4] = Wb[0, :d]
    cols[:, 5] = Wa[0, d:]
    cols[:, 6] = Wb[0, d:]
    return np.ascontiguousarray(
        np.concatenate([W_sa, W_gcnb, cols], axis=1)).astype(np.float32)


def make_core_inputs(x, adj_a, adj_b, R, b_gcnb, n, d, nl, core,
                     np_a=np.float32, np_bc=None):
    if np_bc is None:
        import ml_dtypes
        np_bc = ml_dtypes.bfloat16
    JT, KT, LT = n // 128, d // 128, nl // 128
    rows = np.arange(core * nl, (core + 1) * nl)
    xl = x[rows]
    xt = np.ascontiguousarray(
        xl.reshape(LT, 128, KT, 128).transpose(0, 2, 3, 1))
    adjat = np.ascontiguousarray(adj_a[rows].T).reshape(JT, 128, nl)
    adjbt = np.ascontiguousarray(adj_b[rows].T).reshape(JT, 128, nl)
    return {
        "xt": xt.astype(np_a),
        "rmat": R.reshape(KT, 128, 2 * d + 8).astype(np_a),
        "adjat": adjat.astype(np_bc),
        "adjbt": adjbt.astype(np_bc),
        "bbias": np.ascontiguousarray(
            np.broadcast_to(b_gcnb, (128, d))).astype(np.float32),
        "ident": np.eye(128, dtype=np.float32),
    }


_CACHE = {}


def _install_ntff_hook():
    """Dev-only: register the axon NTFF profile hook so trace=True works."""
    import sys
    import types
    try:
        from antenv import axon_hooks  # noqa: F401
        return
    except ImportError:
        pass
    import antenv
    mod = types.ModuleType("antenv.axon_hooks")
    _h = [None]
    mod.get_axon_ntff_profile_hook = lambda: _h[0]
    mod.set_axon_ntff_profile_hook = lambda hook: _h.__setitem__(0, hook)
    sys.modules["antenv.axon_hooks"] = mod
    antenv.axon_hooks = mod
    from trn_agent_boot.trn_boot import _ntff_profile_via_ctypes
    mod.set_axon_ntff_profile_hook(
        _ntff_profile_via_ctypes("/opt/axon/libaxon_pjrt.so"))


def kernel(x, adj_a, adj_b, W_sa, a_sa, W_gcnb, b_gcnb, Wa, ba, Wb, bb,
           _trace=False, _trace_kwargs=None):
    from concourse.bass_utils import run_bass_kernel_spmd
    if _trace:
        _install_ntff_hook()

    n, d = x.shape
    nl = n // N_CORES
    R = make_r_matrix(W_sa, a_sa, W_gcnb, Wa, Wb, d)

    key = (n, d, nl, float(ba[0]), float(bb[0]))
    if key not in _CACHE:
        _CACHE[key] = build_program(n, d, nl, float(ba[0]), float(bb[0]))
    nc = _CACHE[key]

    in_maps = [make_core_inputs(x, adj_a, adj_b, R, b_gcnb, n, d, nl, c)
               for c in range(N_CORES)]
    res = run_bass_kernel_spmd(nc, in_maps, list(range(N_CORES)),
                               trace=_trace, **(_trace_kwargs or {}))
    out = np.empty((n, d), dtype=np.float32)
    for c in range(N_CORES):
        out[c * nl:(c + 1) * nl] = res.results[c]["out"]
    if _trace:
        kernel._last_results = res
    return out


# revision 6
# speedup vs baseline: 1.4115x; 1.4115x over previous
"""HGCN layer kernel for Trainium2, 8 NeuronCores, row-sharded SPMD.

Reference computation (N=6144, D=512):
    type_sum_a = adj_a @ x ; type_sum_b = adj_b @ x
    attn_a = sigmoid(cat[ts_a, x] @ Wa.T + ba) ; attn_b likewise
    h = x @ W_sa ; s_l = h @ a_sa[:512] ; s_r = h @ a_sa[512:]
    scores[i,j] = s_l[i] + s_r[j]
    e = adj_a * exp(-leaky_relu(scores, 0.01)) ; attn = e / (rowsum(e)+1e-5)
    x_a = attn @ h ; x_b = adj_b @ (x @ W_gcnb) + b_gcnb
    out = sigmoid(attn_a * x_a + attn_b * x_b)

Kernel strategy (per core, NL=768 local rows, global j order everywhere):
  - R = [W_sa | W_gcnb | W_sa@a_l | W_sa@a_r | Wa1.T | Wb1.T | Wa2.T | Wb2.T]
    Phase A computes HX = x_local @ R for LOCAL rows only; one AllGather
    ([h|xw|stats] bf16) replicates every row's h/xW/stats to all cores.
    A tiny barrier AllGather fires at program start so core-launch skew and
    ncfw warmup overlap Phase A instead of the real gather.
  - Gates reassociate (adj@x)@W1.T -> adj@(x@W1.T): the N-vector gate
    contractions (ga, gb) and rowsum(e) ride the main matmuls as N=1
    matmuls against already-loaded weights (adjacency / e tiles).
  - e computed in transposed layout [j(part), i(free)]: lrelu+bias fused
    into one scalar-engine Prelu, exp on scalar, mask-mult on vector.
  - float32r Phase A matmuls; bf16 adjacency/h/xW elsewhere.
"""

import numpy as np
from contextlib import ExitStack

import concourse.bass as bass
import concourse.bacc as bacc
import concourse.mybir as mybir
import concourse.tile as tile

F32 = mybir.dt.float32
F32R = mybir.dt.float32r
BF16 = mybir.dt.bfloat16
AF = mybir.ActivationFunctionType
ALU = mybir.AluOpType

N_CORES = 8


def build_program(n, d, nl, ba, bb, dt_a=F32R, dt_bc=BF16, lrelu_on_act=True):
    """Build the SPMD Bass program. Returns nc."""
    JT = n // 128   # j tiles (contraction/node axis), global order
    LT = nl // 128  # local row tiles
    KT = d // 128   # feature k tiles
    NR = 2 * d + 8  # columns of R
    HX = 2 * d + 8  # gathered row block: h | xw | stats(bf16)
    # stats cols: 0=s_l 1=s_r 2=zero 3=va 4=vb 5=wa2x 6=wb2x 7=pad

    nc = bacc.Bacc("TRN2", target_bir_lowering=False, debug=False,
                   num_devices=N_CORES)

    xt_dram = nc.dram_tensor("xt", [LT, KT, 128, 128], dt_a, kind="ExternalInput")
    r_dram = nc.dram_tensor("rmat", [KT, 128, NR], dt_a, kind="ExternalInput")
    adjat_dram = nc.dram_tensor("adjat", [JT, 128, nl], dt_bc, kind="ExternalInput")
    adjbt_dram = nc.dram_tensor("adjbt", [JT, 128, nl], dt_bc, kind="ExternalInput")
    bbias_dram = nc.dram_tensor("bbias", [128, d], F32, kind="ExternalInput")
    ident_dram = nc.dram_tensor("ident", [128, 128], F32, kind="ExternalInput")
    out_dram = nc.dram_tensor("out", [nl, d], F32, kind="ExternalOutput")

    ag_in = nc.dram_tensor("ag_in", [LT, 128, HX], dt_bc)
    ag_out = nc.dram_tensor("ag_out", [JT, 128, HX], dt_bc, addr_space="Shared")
    dmy_in = nc.dram_tensor("dmy_in", [1, 128, 2], F32)
    dmy_out = nc.dram_tensor("dmy_out", [N_CORES, 128, 2], F32,
                             addr_space="Shared")
    RG = [list(range(N_CORES))]

    def mm(out, lhsT, rhs, start, stop, skip_group_check=False):
        nc.tensor.matmul(out, lhsT, rhs, start=start, stop=stop,
                         skip_group_check=skip_group_check)

    with tile.TileContext(nc) as tc, ExitStack() as ctx:
        const = ctx.enter_context(tc.tile_pool(name="const", bufs=1))

        r_sb = const.tile([128, KT, NR], dt_a, tag="r")
        hx_sb = const.tile([128, JT * HX], dt_bc, tag="hx")
        stats_sb = const.tile([128, JT * 8], F32, tag="stats")
        stats_loc = const.tile([128, LT * 8], F32, tag="statsloc")
        slb_sb = const.tile([128, nl], F32, tag="slb")
        xb_sb = const.tile([128, LT * d], F32, tag="xb")
        xa_sb = const.tile([128, LT * d], F32, tag="xa")
        bbias_sb = const.tile([128, d], F32, tag="bbias")
        ident_sb = const.tile([128, 128], F32, tag="ident")
        ones_bf = const.tile([128, 1], dt_bc, tag="ones_bf")
        ones_row = const.tile([1, 128], F32, tag="ones_r")
        neg1 = const.tile([128, 1], F32, tag="neg1")
        ba_sb = const.tile([128, 1], F32, tag="ba")
        bb_sb = const.tile([128, 1], F32, tag="bb")
        sl_row = const.tile([1, nl], F32, tag="sl_row")
        gate_sb = const.tile([128, 4 * LT], F32, tag="gate")
        dmy_sb = const.tile([128, 2], F32, tag="dmy")
        # gate_sb cols: [0:LT]=recip(rowsum), [LT:2LT]=sig_a, [2LT:3LT]=sig_b,
        # [3LT:4LT]=scratch

        # Barrier collective: syncs cores + warms the CC path during Phase A.
        nc.vector.memset(dmy_sb[:], 1.0)
        nc.gpsimd.dma_start(out=dmy_in[0], in_=dmy_sb[:])
        nc.gpsimd.collective_compute(
            "AllGather", mybir.AluOpType.bypass, replica_groups=RG,
            ins=[dmy_in[:].opt()], outs=[dmy_out[:].opt()])

        for k in range(KT):
            nc.sync.dma_start(out=r_sb[:, k, :], in_=r_dram[k])
        nc.sync.dma_start(out=bbias_sb[:], in_=bbias_dram[:])
        nc.sync.dma_start(out=ident_sb[:], in_=ident_dram[:])
        nc.vector.memset(ones_bf[:], 1.0)
        nc.vector.memset(ones_row[:], 1.0)
        nc.vector.memset(neg1[:], -1.0)
        nc.vector.memset(ba_sb[:], float(ba))
        nc.vector.memset(bb_sb[:], float(bb))

        # ---- Phase A: HX = x_local @ R (local rows only) ----
        with tc.tile_pool(name="xt_pool", bufs=3) as xtp, \
             tc.tile_pool(name="hx_out", bufs=3) as hxop, \
             tc.tile_pool(name="psA", bufs=2, space="PSUM") as psA:
            for m in range(LT):
                xt_t = xtp.tile([128, KT * 128], dt_a, tag="xt")
                for k in range(KT):
                    nc.sync.dma_start(out=xt_t[:, k * 128:(k + 1) * 128],
                                      in_=xt_dram[m, k])
                ph = psA.tile([128, d], F32, tag="ph")
                pw = psA.tile([128, d], F32, tag="pw")
                ps = psA.tile([128, 8], F32, tag="ps")
                for k in range(KT):
                    lhsT = xt_t[:, k * 128:(k + 1) * 128]
                    st, sp = (k == 0), (k == KT - 1)
                    mm(ph[:], lhsT, r_sb[:, k, 0:d], st, sp)
                    mm(pw[:], lhsT, r_sb[:, k, d:2 * d], st, sp)
                    mm(ps[:], lhsT, r_sb[:, k, 2 * d:NR], st, sp)
                hx_t = hxop.tile([128, HX], dt_bc, tag="hxt")
                nc.scalar.copy(hx_t[:, 0:d], ph[:])
                nc.scalar.copy(hx_t[:, d:2 * d], pw[:])
                nc.vector.tensor_copy(hx_t[:, 2 * d:HX], ps[:])
                nc.vector.tensor_copy(stats_loc[:, m * 8:(m + 1) * 8], ps[:])
                nc.gpsimd.dma_start(out=ag_in[m], in_=hx_t[:])

        # ---- AllGather h|xw|stats (bf16) across the 8 cores ----
        nc.gpsimd.collective_compute(
            "AllGather", mybir.AluOpType.bypass, replica_groups=RG,
            ins=[ag_in[:].opt()], outs=[ag_out[:].opt()])

        # Readback on the scalar queue (sync queue carries adjacency).
        for j in range(JT):
            nc.scalar.dma_start(out=hx_sb[:, j * HX:(j + 1) * HX],
                                in_=ag_out[j])

        # ---- Phase A2: build SL broadcast [128, nl] from local s_l ----
        # (local stats only; runs during the AllGather)
        with tc.tile_pool(name="psA2", bufs=1, space="PSUM") as psA2:
            chunks = [(o, min(512, nl - o)) for o in range(0, nl, 512)]
            ptrs = [psA2.tile([1, w], F32, tag=f"psl{ci}", name=f"psl{ci}")
                    for ci, (o, w) in enumerate(chunks)]
            for t in range(LT):
                ci, off = divmod(t * 128, 512)
                mm(ptrs[ci][0:1, off:off + 128],
                   stats_loc[:, t * 8:t * 8 + 1], ident_sb[:], True, True)
            for ci, (o, w) in enumerate(chunks):
                nc.vector.tensor_copy(sl_row[0:1, o:o + w], ptrs[ci][0:1, :])
            for ci, (o, w) in enumerate(chunks):
                pb = psA2.tile([128, w], F32, tag="pslb")
                mm(pb[:], ones_row[:], sl_row[0:1, o:o + w], True, True)
                nc.vector.tensor_copy(slb_sb[:, o:o + w], pb[:])

        # ---- Phase B: x_b = adj_b @ xW ; gb as N=1 rides on loaded weights ----
        with tc.tile_pool(name="adjB", bufs=5) as adjp, \
             tc.tile_pool(name="psB", bufs=1, space="PSUM") as psB:
            pb_acc = [psB.tile([128, d], F32, tag=f"pb{i}", name=f"pb{i}")
                      for i in range(LT)]
            pgbT = psB.tile([128, d], F32, tag="pgbT")
            for j in range(JT):
                at = adjp.tile([128, nl], dt_bc, tag="adj")
                nc.sync.dma_start(out=at[:], in_=adjbt_dram[j])
                xw_j = hx_sb[:, j * HX + d:j * HX + 2 * d]
                vb_j = hx_sb[:, j * HX + 2 * d + 4:j * HX + 2 * d + 5]
                st, sp = (j == 0), (j == JT - 1)
                for i in range(LT):
                    ai = at[:, i * 128:(i + 1) * 128]
                    mm(pb_acc[i][:], ai, xw_j, st, sp)
                    mm(pgbT[:, i:i + 1], ai, vb_j, st and i == 0,
                       sp and i == LT - 1, skip_group_check=True)
            for i in range(LT):
                nc.scalar.copy(xb_sb[:, i * d:(i + 1) * d], pb_acc[i][:])
            nc.vector.tensor_copy(gate_sb[:, 2 * LT:3 * LT], pgbT[:, 0:LT])

        # ---- Phase C: e = adj_a * exp(-lrelu(s)); x_a = e^T.T @ h ----
        with tc.tile_pool(name="adjC", bufs=5) as adjp, \
             tc.tile_pool(name="ewC", bufs=3) as ewp, \
             tc.tile_pool(name="psC", bufs=1, space="PSUM") as psC:
            pc_acc = [psC.tile([128, d], F32, tag=f"pc{i}", name=f"pc{i}")
                      for i in range(LT)]
            prsT = psC.tile([128, d], F32, tag="prsT")
            pgaT = psC.tile([128, d], F32, tag="pgaT")
            for j in range(JT):
                at = adjp.tile([128, nl], dt_bc, tag="adj")
                nc.sync.dma_start(out=at[:], in_=adjat_dram[j])
                # s_r (f32) for the activation bias
                nc.vector.tensor_copy(stats_sb[:, j * 8:(j + 1) * 8],
                                      hx_sb[:, j * HX + 2 * d:(j + 1) * HX])
                s_r = stats_sb[:, j * 8 + 1:j * 8 + 2]
                m_t = ewp.tile([128, nl], F32, tag="m")
                if lrelu_on_act:
                    nc.scalar.activation(m_t[:], slb_sb[:], AF.Prelu,
                                         bias=s_r, alpha=0.01)
                else:
                    nc.vector.tensor_scalar_add(m_t[:], slb_sb[:], s_r)
                    nc.vector.scalar_tensor_tensor(m_t[:], m_t[:], 0.01, m_t[:],
                                                   op0=ALU.mult, op1=ALU.max)
                nc.scalar.activation(m_t[:], m_t[:], AF.Exp, scale=neg1[:])
                e_t = ewp.tile([128, nl], dt_bc, tag="e")
                nc.vector.tensor_tensor(e_t[:], m_t[:], at[:], op=ALU.mult)
                h_j = hx_sb[:, j * HX:j * HX + d]
                va_j = hx_sb[:, j * HX + 2 * d + 3:j * HX + 2 * d + 4]
                st, sp = (j == 0), (j == JT - 1)
                for i in range(LT):
                    ei = e_t[:, i * 128:(i + 1) * 128]
                    mm(pc_acc[i][:], ei, h_j, st, sp)
                    mm(prsT[:, i:i + 1], ei, ones_bf[:], st and i == 0,
                       sp and i == LT - 1, skip_group_check=True)
                for i in range(LT):
                    mm(pgaT[:, i:i + 1], at[:, i * 128:(i + 1) * 128], va_j,
                       st and i == 0, sp and i == LT - 1,
                       skip_group_check=True)
            for i in range(LT):
                nc.scalar.copy(xa_sb[:, i * d:(i + 1) * d], pc_acc[i][:])
            nc.vector.tensor_copy(gate_sb[:, 3 * LT:4 * LT], prsT[:, 0:LT])
            nc.vector.tensor_copy(gate_sb[:, LT:2 * LT], pgaT[:, 0:LT])

        # ---- Phase D: gates + combine ----
        with tc.tile_pool(name="outD", bufs=2) as outp:
            for i in range(LT):
                # recip(rowsum + 1e-5)
                nc.vector.tensor_scalar_add(gate_sb[:, 3 * LT + i:3 * LT + i + 1],
                                            gate_sb[:, 3 * LT + i:3 * LT + i + 1],
                                            1e-5)
                nc.vector.reciprocal(gate_sb[:, i:i + 1],
                                     gate_sb[:, 3 * LT + i:3 * LT + i + 1])
                # sig_a = sigmoid(ga + wa2x + ba)
                nc.vector.tensor_tensor(gate_sb[:, LT + i:LT + i + 1],
                                        gate_sb[:, LT + i:LT + i + 1],
                                        stats_loc[:, i * 8 + 5:i * 8 + 6],
                                        op=ALU.add)
                nc.scalar.activation(gate_sb[:, LT + i:LT + i + 1],
                                     gate_sb[:, LT + i:LT + i + 1],
                                     AF.Sigmoid, bias=ba_sb[:])
                # sig_b = sigmoid(gb + wb2x + bb)
                nc.vector.tensor_tensor(gate_sb[:, 2 * LT + i:2 * LT + i + 1],
                                        gate_sb[:, 2 * LT + i:2 * LT + i + 1],
                                        stats_loc[:, i * 8 + 6:i * 8 + 7],
                                        op=ALU.add)
                nc.scalar.activation(gate_sb[:, 2 * LT + i:2 * LT + i + 1],
                                     gate_sb[:, 2 * LT + i:2 * LT + i + 1],
                                     AF.Sigmoid, bias=bb_sb[:])
            for i in range(LT):
                u_t = outp.tile([128, d], F32, tag="u")
                # u = sig_a * (x_a_raw * recip)
                nc.vector.tensor_scalar(u_t[:], xa_sb[:, i * d:(i + 1) * d],
                                        gate_sb[:, i:i + 1],
                                        gate_sb[:, LT + i:LT + i + 1],
                                        op0=ALU.mult, op1=ALU.mult)
                t_t = outp.tile([128, d], F32, tag="t")
                # t = x_b_raw + b_gcnb
                nc.vector.tensor_tensor(t_t[:], xb_sb[:, i * d:(i + 1) * d],
                                        bbias_sb[:], op=ALU.add)
                # y = sigmoid(t * sig_b + u)
                nc.vector.scalar_tensor_tensor(t_t[:], t_t[:],
                                               gate_sb[:, 2 * LT + i:2 * LT + i + 1],
                                               u_t[:], op0=ALU.mult, op1=ALU.add)
                y_t = outp.tile([128, d], F32, tag="y")
                nc.scalar.activation(y_t[:], t_t[:], AF.Sigmoid)
                nc.sync.dma_start(out=out_dram[i * 128:(i + 1) * 128, :],
                                  in_=y_t[:])

    nc.compile()
    return nc


def make_r_matrix(W_sa, a_sa, W_gcnb, Wa, Wb, d):
    cols = np.zeros((d, 8), dtype=np.float32)
    cols[:, 0] = W_sa @ a_sa[0, :d]
    cols[:, 1] = W_sa @ a_sa[0, d:]
    # col 2 stays zero
    cols[:, 3] = Wa[0, :d]
    cols[:, 4] = Wb[0, :d]
    cols[:, 5] = Wa[0, d:]
    cols[:, 6] = Wb[0, d:]
    return np.ascontiguousarray(
        np.concatenate([W_sa, W_gcnb, cols], axis=1)).astype(np.float32)


def make_core_inputs(x, adj_a, adj_b, R, b_gcnb, n, d, nl, core,
                     np_a=np.float32, np_bc=None):
    if np_bc is None:
        import ml_dtypes
        np_bc = ml_dtypes.bfloat16
    JT, KT, LT = n // 128, d // 128, nl // 128
    rows = np.arange(core * nl, (core + 1) * nl)
    xl = x[rows]
    xt = np.ascontiguousarray(
        xl.reshape(LT, 128, KT, 128).transpose(0, 2, 3, 1))
    adjat = np.ascontiguousarray(adj_a[rows].T).reshape(JT, 128, nl)
    adjbt = np.ascontiguousarray(adj_b[rows].T).reshape(JT, 128, nl)
    return {
        "xt": xt.astype(np_a),
        "rmat": R.reshape(KT, 128, 2 * d + 8).astype(np_a),
        "adjat": adjat.astype(np_bc),
        "adjbt": adjbt.astype(np_bc),
        "bbias": np.ascontiguousarray(
            np.broadcast_to(b_gcnb, (128, d))).astype(np.float32),
        "ident": np.eye(128, dtype=np.float32),
    }


_CACHE = {}


def _install_ntff_hook():
    """Dev-only: register the axon NTFF profile hook so trace=True works."""
    import sys
    import types
    try:
        from antenv import axon_hooks  # noqa: F401
        return
    except ImportError:
        pass
    import antenv
    mod = types.ModuleType("antenv.axon_hooks")
    _h = [None]
    mod.get_axon_ntff_profile_hook = lambda: _h[0]
    mod.set_axon_ntff_profile_hook = lambda hook: _h.__setitem__(0, hook)
    sys.modules["antenv.axon_hooks"] = mod
    antenv.axon_hooks = mod
    from trn_agent_boot.trn_boot import _ntff_profile_via_ctypes
    mod.set_axon_ntff_profile_hook(
        _ntff_profile_via_ctypes("/opt/axon/libaxon_pjrt.so"))


def kernel(x, adj_a, adj_b, W_sa, a_sa, W_gcnb, b_gcnb, Wa, ba, Wb, bb,
           _trace=False, _trace_kwargs=None):
    from concourse.bass_utils import run_bass_kernel_spmd
    if _trace:
        _install_ntff_hook()

    n, d = x.shape
    nl = n // N_CORES
    R = make_r_matrix(W_sa, a_sa, W_gcnb, Wa, Wb, d)

    key = (n, d, nl, float(ba[0]), float(bb[0]))
    if key not in _CACHE:
        _CACHE[key] = build_program(n, d, nl, float(ba[0]), float(bb[0]))
    nc = _CACHE[key]

    in_maps = [make_core_inputs(x, adj_a, adj_b, R, b_gcnb, n, d, nl, c)
               for c in range(N_CORES)]
    res = run_bass_kernel_spmd(nc, in_maps, list(range(N_CORES)),
                               trace=_trace, **(_trace_kwargs or {}))
    out = np.empty((n, d), dtype=np.float32)
    for c in range(N_CORES):
        out[c * nl:(c + 1) * nl] = res.results[c]["out"]
    if _trace:
        kernel._last_results = res
    return out
